# revision 2
# baseline (speedup 1.0000x reference)
"""Trainium2 Bass kernel for nn_PerformerAttention.

reference math (B,H,S,D = 4,8,2048,64):
    qf = relu(q @ W.T); kf = relu(k @ W.T)          # [B,H,S,D]
    scores = qf @ kf.T                              # [B,H,S,S]
    attn_weights = softmax(scores, axis=-1)
    attn_output  = v * rowsum(attn_weights) == v    # softmax rows sum to 1
    returns (attn_output, attn_weights)

Sharding: B*H = 32 (b,h) pairs, 4 per core across 8 cores.  Host-side
layout prep only: q/k are transposed to [.., D, S] so the device never
needs on-chip transposes (matmul contracts over the partition dim).

Per-core device program (per head):
    qfT = relu(W^T.T @ qT)  kfT likewise      # feature transform, [64, S]
          (written to both SBUF partition halves via PE column tiling,
           so score matmuls can row-pack the half-empty K=64 PE array)
    for each 128-row tile of scores:
        s  = qfT_chunk.T @ kfT               # PSUM [128, S] fp32
        neg/max, exp(s - max) with fused row-sum, multiply by 1/sum
        DMA out
"""

import os
import numpy as np

B, H, S, D = 4, 8, 2048, 64
NCORES = 8
HPC = (B * H) // NCORES      # heads per core = 4
PAIRS = HPC // 2             # head pairs per core = 2
RT = S // 128                # 128-row score tiles per head = 16
NCH = S // 512               # 512-col matmul chunks per row tile = 4

# Tunables (test.py may override before calling kernel()).
CONFIG = {
    "pack": True,        # row-pack score matmuls across PE partition halves
    "copymax": True,     # fused PSUM->SBUF copy + row-max on VectorE
    "smul_frac": 0.45,   # fraction of normalize-muls issued on ScalarE
    "trace": False,      # request NTFF profile from the run
}

_CACHE = {}


def _build_program(cfg):
    from contextlib import ExitStack

    import concourse.bacc as bacc
    import concourse.mybir as mybir
    import concourse.tile as tile

    f32 = mybir.dt.float32
    AF = mybir.ActivationFunctionType
    OP = mybir.AluOpType
    AX = mybir.AxisListType

    nc = bacc.Bacc(
        "TRN2",
        target_bir_lowering=False,
        debug=False,
        num_devices=NCORES,
    )

    qt = nc.dram_tensor("qt", [PAIRS * 128, S], f32, kind="ExternalInput").ap()
    kt = nc.dram_tensor("kt", [PAIRS * 128, S], f32, kind="ExternalInput").ap()
    wt = nc.dram_tensor("wt", [128, D], f32, kind="ExternalInput").ap()
    out = nc.dram_tensor("out", [HPC * S, S], f32, kind="ExternalOutput").ap()

    n_tiles = HPC * RT
    n_smul = int(round(cfg["smul_frac"] * n_tiles))
    # Spread the ScalarE-muls evenly over the tile sequence.
    smul_every = n_tiles / max(n_smul, 1)

    with tile.TileContext(nc) as tc, ExitStack() as ctx:
        const = ctx.enter_context(tc.tile_pool(name="const", bufs=1))
        inp = ctx.enter_context(tc.tile_pool(name="inp", bufs=2))
        feat = ctx.enter_context(tc.tile_pool(name="feat", bufs=2))
        psum = ctx.enter_context(tc.tile_pool(name="psum", bufs=2, space="PSUM"))
        work = ctx.enter_context(tc.tile_pool(name="work", bufs=3))
        stat = ctx.enter_context(tc.tile_pool(name="stat", bufs=8))
        outp = ctx.enter_context(tc.tile_pool(name="outp", bufs=4))

        wtt = const.tile([128, D], f32, tag="wtt")
        nc.sync.dma_start(wtt[:], wt[:, :])

        tile_idx = 0
        smul_emitted = 0

        def softmax_tail(ps, h, m):
            """PSUM scores tile -> normalized SBUF tile -> DMA out."""
            nonlocal tile_idx, smul_emitted
            negmax = stat.tile([128, 1], f32, tag="negmax")
            rowsum = stat.tile([128, 1], f32, tag="rowsum")
            rinv = stat.tile([128, 1], f32, tag="rinv")
            expt = work.tile([128, S], f32, tag="expt")
            if cfg["copymax"]:
                # sc = -scores (SBUF copy), negmax = min(-scores) = -rowmax.
                # Frees the PSUM tile after this single VectorE pass.
                sc = work.tile([128, S], f32, tag="sc")
                nc.vector.tensor_scalar(
                    sc[:], ps[:], -1.0, None, OP.mult, OP.min, accum_out=negmax[:]
                )
                # exp(-1*sc + negmax) = exp(s - max); fused row-sum.
                nc.scalar.activation(
                    expt[:], sc[:], AF.Exp,
                    bias=negmax[:], scale=-1.0, accum_out=rowsum[:],
                )
            else:
                nc.vector.reduce_max(negmax[:], ps[:], AX.X, negate=True)
                nc.scalar.activation(
                    expt[:], ps[:], AF.Exp,
                    bias=negmax[:], scale=1.0, accum_out=rowsum[:],
                )
            nc.vector.reciprocal(rinv[:], rowsum[:])
            ot = outp.tile([128, S], f32, tag="ot")
            want_smul = smul_emitted < int((tile_idx + 1) / smul_every + 1e-9)
            if want_smul and smul_emitted < n_smul:
                nc.scalar.activation(ot[:], expt[:], AF.Copy, bias=0.0, scale=rinv[:])
                smul_emitted += 1
            else:
                nc.vector.tensor_scalar(ot[:], expt[:], rinv[:], None, OP.mult, OP.bypass)
            nc.sync.dma_start(out[h * S + 128 * m : h * S + 128 * (m + 1), :], ot[:])
            tile_idx += 1

        for p in range(PAIRS):
            qtt = inp.tile([128, S], f32, tag="qtt")
            nc.sync.dma_start(qtt[:], qt[128 * p : 128 * (p + 1), :])
            ktt = inp.tile([128, S], f32, tag="ktt")
            nc.sync.dma_start(ktt[:], kt[128 * p : 128 * (p + 1), :])
            for e in range(2):
                h = 2 * p + e
                rb = 64 * e  # partition base of this head's qT/kT rows
                # Feature transform: qfT/kfT duplicated across both
                # partition halves (PE column tiling writes each half).
                qf = feat.tile([128, S], f32, tag="qf")
                kf = feat.tile([128, S], f32, tag="kf")
                for src, dst in ((qtt, qf), (ktt, kf)):
                    pf = psum.tile([128, S], f32, tag="ps")
                    for j in range(NCH):
                        cs = slice(512 * j, 512 * (j + 1))
                        for c in (0, 64):
                            nc.tensor.matmul(
                                pf[c : c + 64, cs],
                                lhsT=wtt[rb : rb + 64, :],
                                rhs=src[rb : rb + 64, cs],
                                start=True, stop=True,
                                tile_position=(rb, c),
                            )
                    nc.scalar.activation(dst[:], pf[:], AF.Relu)

                if cfg["pack"]:
                    # Interleave two row tiles on opposite PE halves so the
                    # K=64 matmuls run concurrently (distinct row groups).
                    for mp in range(RT // 2):
                        m0, m1 = 2 * mp, 2 * mp + 1
                        ps0 = psum.tile([128, S], f32, tag="ps")
                        ps1 = psum.tile([128, S], f32, tag="ps")
                        for j in range(NCH):
                            cs = slice(512 * j, 512 * (j + 1))
                            nc.tensor.matmul(
                                ps0[:, cs],
                                lhsT=qf[0:64, 128 * m0 : 128 * (m0 + 1)],
                                rhs=kf[0:64, cs],
                                start=True, stop=True,
                                tile_position=(0, 0),
                            )
                            nc.tensor.matmul(
                                ps1[:, cs],
                                lhsT=qf[64:128, 128 * m1 : 128 * (m1 + 1)],
                                rhs=kf[64:128, cs],
                                start=True, stop=True,
                                tile_position=(64, 0),
                            )
                        softmax_tail(ps0, h, m0)
                        softmax_tail(ps1, h, m1)
                else:
                    for m in range(RT):
                        ps = psum.tile([128, S], f32, tag="ps")
                        for j in range(NCH):
                            cs = slice(512 * j, 512 * (j + 1))
                            nc.tensor.matmul(
                                ps[:, cs],
                                lhsT=qf[0:64, 128 * m : 128 * (m + 1)],
                                rhs=kf[0:64, cs],
                                start=True, stop=True,
                                tile_position=(0, 0),
                            )
                        softmax_tail(ps, h, m)

    nc.compile()
    return nc


def _get_program(cfg):
    key = (cfg["pack"], cfg["copymax"], cfg["smul_frac"])
    if key not in _CACHE:
        _CACHE[key] = _build_program(cfg)
    return _CACHE[key]


def make_in_maps(q, k, random_weights):
    """Host-side sharding/layout prep -> per-core input dicts."""
    q = np.asarray(q, dtype=np.float32)
    k = np.asarray(k, dtype=np.float32)
    w = np.asarray(random_weights, dtype=np.float32)
    # [B,H,S,D] -> [B*H, D, S]
    qT = np.ascontiguousarray(q.transpose(0, 1, 3, 2)).reshape(B * H, D, S)
    kT = np.ascontiguousarray(k.transpose(0, 1, 3, 2)).reshape(B * H, D, S)
    wt = np.ascontiguousarray(np.concatenate([w.T, w.T], axis=0))  # [128, D]
    in_maps = []
    for c in range(NCORES):
        qc = np.ascontiguousarray(qT[HPC * c : HPC * (c + 1)]).reshape(PAIRS * 128, S)
        kc = np.ascontiguousarray(kT[HPC * c : HPC * (c + 1)]).reshape(PAIRS * 128, S)
        in_maps.append({"qt": qc, "kt": kc, "wt": wt})
    return in_maps


def run_device(q, k, random_weights, cfg=None, trace=None):
    """Compile (cached), run on all 8 cores, return (attn_weights, results)."""
    from concourse.bass_utils import run_bass_kernel_spmd

    cfg = dict(CONFIG if cfg is None else cfg)
    if trace is not None:
        cfg["trace"] = trace
    nc = _get_program(cfg)
    in_maps = make_in_maps(q, k, random_weights)
    res = run_bass_kernel_spmd(
        nc, in_maps, core_ids=list(range(NCORES)), trace=cfg["trace"]
    )
    outs = [res.results[c]["out"].reshape(HPC, S, S) for c in range(NCORES)]
    attn_weights = np.concatenate(outs, axis=0).reshape(B, H, S, S)
    return attn_weights, res


def kernel(q, k, v, random_weights):
    attn_weights, _ = run_device(q, k, random_weights)
    attn_output = np.asarray(v, dtype=np.float32)
    return attn_output, attn_weights
